# revision 35
# baseline (speedup 1.0000x reference)
"""DgCD forward (topk channel masking) on 8 Trainium2 NeuronCores.

v3: fully sharded middle + SBUF-cached x + fused combine-broadcast matmuls.
  - Phase A: per-row channel-block loads ([128p, 16*196], contiguous 12.5KB
    lines), avg-pool reduce, bf16 x-cache in SBUF.
  - Middle: each core computes scores/top-k only for its own 16 batch rows in
    the packed [(16b x 8q), 256] layout; cross-core coupling via 4 small
    collectives (env-pair sums, total sums, gram diag, mask count).
  - Row-wide sums use one EE=E16c@E16b matmul (combine+broadcast fused); the
    top-k search keeps lo/hi replicated per partition so each round is one
    matmul plus vector work. Softmax skips max-subtraction (z standardized).
  - Phase C: mask-multiply from the bf16 cache, store only (no x re-read).
Channel order in the middle is block-permuted (c' = (c%16)*128 + c//16); all
middle math is channel-permutation-equivariant and phase C maps the mask back.
"""
import os
import sys
sys.path.insert(0, "/opt/trn_rl_repo")
import numpy as np
from contextlib import ExitStack

import concourse.bass as bass
import concourse.bacc as bacc_mod
import concourse.mybir as mybir
import concourse.tile as tile
from concourse.bass_utils import run_bass_kernel_spmd

f32 = mybir.dt.float32
bf16 = mybir.dt.bfloat16
u8 = mybir.dt.uint8
AL = mybir.AluOpType
AF = mybir.ActivationFunctionType
AX = mybir.AxisListType

B, C, HW = 128, 2048, 196
NCORES = 8
BL = B // NCORES          # 16 batch rows per core
NJ = 16                   # sub-channels per partition block
NQ = 8                    # 256-wide packed chunks per row
FREE_B = NJ * HW          # 3136 floats per partition per row
NMID = 7                  # thresholds probed per search round
SEARCH_ROUNDS = 6         # 8^6 = 2^18 bracket shrink
LO0 = -104.0

C196 = float(np.float32(1.0 / 196.0))
C31 = float(np.float32(1.0 / 31.0))
C127 = float(np.float32(1.0 / 127.0))

_CACHE = {}
LAST = {}


def _consts():
    ident = np.eye(128, dtype=np.float32)
    E16b = np.zeros((16, 128), np.float32)   # [16,1] row vals -> [128,1] bcast
    for p in range(128):
        E16b[p // 8, p] = 1.0
    EE = np.zeros((128, 128), np.float32)    # row-sum + bcast in one matmul
    for kk in range(128):
        for p in range(128):
            if kk // 8 == p // 8:
                EE[kk, p] = 1.0
    Eh0 = np.zeros((16, 128), np.float32)    # statT [16,128] -> packed halves
    Eh1 = np.zeros((16, 128), np.float32)
    for p in range(128):
        Eh0[2 * (p % 8), p] = 1.0
        Eh1[2 * (p % 8) + 1, p] = 1.0
    E8s = np.zeros((128, 8), np.float32)     # sum over b for fixed q
    E8b = np.zeros((8, 128), np.float32)     # [8,256] chunk stats -> [128,256]
    for p in range(128):
        E8s[p, p % 8] = 1.0
        E8b[p % 8, p] = 1.0
    ones1 = np.ones((1, 128), np.float32)
    ones128 = np.ones((128, 1), np.float32)
    K7r = np.zeros((128, NMID), np.float32)
    for i in range(NMID):
        K7r[:, i] = float(i + 1)
    return {"ident": ident, "E16b": E16b, "EE": EE, "Eh0": Eh0, "Eh1": Eh1,
            "E8s": E8s, "E8b": E8b, "ones1": ones1, "ones128": ones128,
            "K7r": K7r}


def build(k, rho):
    nc = bacc_mod.Bacc()
    x_d = nc.dram_tensor("x", [BL, C, HW], f32, kind="ExternalInput")
    r_d = nc.dram_tensor("r", [BL, C], f32, kind="ExternalInput")
    envm_d = nc.dram_tensor("envm", [128, 256], f32, kind="ExternalInput")
    cd = {n: nc.dram_tensor(n, list(v.shape), f32, kind="ExternalInput")
          for n, v in _consts().items()}
    out_d = nc.dram_tensor("out", [BL, C, HW], f32, kind="ExternalOutput")

    RHO = float(np.float32(rho))
    KF = float(k)

    with tile.TileContext(nc) as tc, ExitStack() as ctx:
        pool = ctx.enter_context(tc.tile_pool(name="main", bufs=1))
        big = ctx.enter_context(tc.tile_pool(name="bigp", bufs=1))
        psum = ctx.enter_context(tc.tile_pool(name="psum", bufs=1, space="PSUM"))
        dram = ctx.enter_context(tc.tile_pool(name="dram", bufs=1, space="DRAM"))
        xpool = ctx.enter_context(tc.tile_pool(name="xio", bufs=4))

        _n = iter(range(100000))

        def psB(shape):
            return psum.tile(shape, f32, tag="psB", bufs=3,
                             name=f"psB_{next(_n)}", padded_shape=[128, 256])

        def psC(shape):
            return psum.tile(shape, f32, tag="psC", bufs=4,
                             name=f"psC_{next(_n)}", padded_shape=[128, 1])

        def mid(tag, shape=None, dt=f32):
            return pool.tile(shape or [128, 256], dt, tag=tag,
                             name=f"{tag}_{next(_n)}")

        # ---- constants (issued after phase-A loads; not needed until then) ----
        cs = {}
        for n, v in _consts().items():
            cs[n] = pool.tile(list(v.shape), f32, tag="c_" + n, name="c_" + n)

        def sbuf_copy(ps, tag, bufs=1):
            t = pool.tile([ps.shape[0], ps.shape[1]], f32, tag=tag,
                          name=f"sc_{tag}_{next(_n)}", bufs=bufs)
            nc.scalar.copy(t[:], ps[:])
            return t

        def ee_bcast(val_pp):
            """[128,1] per-partition -> per-row sums broadcast [128,1] PSUM."""
            ps = psC([128, 1])
            nc.tensor.matmul(ps[:], cs["EE"][:], val_pp[:], start=True,
                             stop=True)
            return ps

        # =========== PHASE A: load x, avg-pool, cache bf16 ===========
        r_s = pool.tile([BL, C], f32, tag="rp16", bufs=2, name="r_s")
        nc.sync.dma_start(r_s[:], r_d[:])

        cache = big.tile([128, BL * FREE_B], bf16, tag="xcache")
        avgw = pool.tile([128, NJ * BL], f32, tag="avgw")   # free = (jj, b)
        HFREE = FREE_B // 2
        xv = x_d.rearrange("b (p jj) h -> b p (jj h)", p=128)
        nhalf = 0
        for b in range(BL):
            for h in range(2):
                xt = xpool.tile([128, HFREE], f32, tag="xa")
                src = xv[b, :, h * HFREE:(h + 1) * HFREE]
                if nhalf % 2 == 0:
                    nc.sync.dma_start(xt[:], src)
                else:
                    nc.gpsimd.dma_start(xt[:], src)
                nhalf += 1
                nc.vector.reduce_sum(
                    avgw.rearrange("p (jj b) -> p jj b", b=BL)
                    [:, h * 8:(h + 1) * 8, b],
                    xt.rearrange("p (jj hh) -> p jj hh", jj=NJ // 2)[:],
                    axis=AX.X)
                nc.scalar.activation(
                    cache[:, b * FREE_B + h * HFREE:b * FREE_B + (h + 1) * HFREE],
                    xt[:], AF.Copy)

        # ---- consts + envm load (after loads are queued) ----
        for n in cs:
            nc.sync.dma_start(cs[n][:], cd[n][:])
        ident = cs["ident"]
        envm = pool.tile([128, 256], f32, tag="envm")
        nc.sync.dma_start(envm[:], envm_d[:])

        # ---- per-channel batch sums -> AllGather (faster than AllReduce) ----
        nc.vector.tensor_scalar(avgw[:], avgw[:], C196, None, AL.mult)
        sqw = pool.tile([128, NJ * BL], f32, tag="sqw")
        nc.vector.tensor_mul(sqw[:], avgw[:], avgw[:])
        st = pool.tile([128, 32], f32, tag="st")
        nc.vector.reduce_sum(st[:, 0:16],
                             avgw.rearrange("p (jj b) -> p jj b", b=BL)[:],
                             axis=AX.X)
        nc.vector.reduce_sum(st[:, 16:32],
                             sqw.rearrange("p (jj b) -> p jj b", b=BL)[:],
                             axis=AX.X)
        st_in = dram.tile([128, 32], f32, tag="st_in")
        nc.sync.dma_start(st_in[:], st[:])
        st_out_d = dram.tile([NCORES, 128, 32], f32, tag="st_out_d")
        nc.gpsimd.collective_compute(
            "AllGather", AL.bypass, replica_groups=[list(range(NCORES))],
            ins=[st_in.opt()], outs=[st_out_d.opt()])

        # ---- r -> permuted packed + ln(r)  (overlaps collectives) ----
        r_rp = pool.tile([BL, C], f32, tag="rp16", bufs=2, name="r_rp")
        nc.vector.tensor_copy(r_rp.rearrange("b (jj p) -> b jj p", jj=NJ)[:],
                              r_s.rearrange("b (p jj) -> b jj p", jj=NJ)[:])
        r_rt = dram.tile([BL, C], f32, tag="r_rt")
        nc.sync.dma_start(r_rt[:], r_rp[:])
        r_pk = mid("r_pk")
        nc.sync.dma_start(r_pk[:], r_rt.rearrange("b (q j) -> (b q) j", q=NQ)[:])
        lnr = mid("lnr")
        nc.scalar.activation(lnr[:], r_pk[:], AF.Ln)

        # ---- avg -> row-permuted -> packed  (overlaps collectives) ----
        avg_rp = pool.tile([BL, C], f32, tag="rp16", bufs=2, name="avg_rp")
        for jj in range(NJ):
            tp = psB([BL, 128])
            nc.tensor.transpose(tp[:], avgw[:, jj * BL:(jj + 1) * BL], ident[:])
            nc.vector.tensor_copy(avg_rp[:, jj * 128:(jj + 1) * 128], tp[:])
        avg_rt = dram.tile([BL, C], f32, tag="avg_rt")
        nc.sync.dma_start(avg_rt[:], avg_rp[:])
        avg_pk = mid("avg_pk")
        nc.sync.dma_start(avg_pk[:],
                          avg_rt.rearrange("b (q j) -> (b q) j", q=NQ)[:])

        # =========== stats -> z ===========
        st_all = pool.tile([128, NCORES * 32], f32, tag="st_all")
        nc.sync.dma_start(st_all.rearrange("p (r s) -> p r s", r=NCORES)[:],
                          st_out_d.rearrange("r p s -> p r s")[:])
        st_tot = pool.tile([128, 32], f32, tag="st_tot")
        nc.vector.reduce_sum(st_tot[:],
                             st_all.rearrange("p (r s) -> p s r", r=NCORES)[:],
                             axis=AX.X)
        envp = pool.tile([128, NCORES * 32], f32, tag="envp")
        nc.vector.tensor_mul(envp[:], st_all[:], envm[:])
        st_env = pool.tile([128, 32], f32, tag="st_env")
        nc.vector.reduce_sum(st_env[:],
                             envp.rearrange("p (r s) -> p s r", r=NCORES)[:],
                             axis=AX.X)

        stats4 = pool.tile([128, 64], f32, tag="stats4")  # m_e|rsd_e|m_t|rsd_t

        def mk_stats(src, dst_m, dst_r, n, cinv):
            nc.vector.tensor_scalar(dst_m, src[:, 0:16], 1.0 / n, None, AL.mult)
            t = mid("vtmp", [128, 16])
            nc.vector.tensor_mul(t[:], dst_m, dst_m)
            nc.vector.tensor_scalar(t[:], t[:], float(n), None, AL.mult)
            v = mid("vvar", [128, 16])
            nc.vector.tensor_sub(v[:], src[:, 16:32], t[:])
            nc.vector.tensor_scalar(v[:], v[:], cinv, 1e-05, AL.mult, op1=AL.add)
            sd = mid("vsd", [128, 16])
            nc.scalar.activation(sd[:], v[:], AF.Sqrt)
            nc.vector.reciprocal(dst_r, sd[:])

        mk_stats(st_env, stats4[:, 0:16], stats4[:, 16:32], 32, C31)
        mk_stats(st_tot, stats4[:, 32:48], stats4[:, 48:64], 128, C127)

        def bcast_pk(src16, tag):
            """[128,16] per-(p,jj) channel stat -> packed [128,256] bcast."""
            tp = psB([16, 128])
            nc.tensor.transpose(tp[:], src16, ident[:])
            sT = sbuf_copy(tp, "sT_" + tag)
            ps = psB([128, 256])
            nc.tensor.matmul(ps[:, 0:128], cs["Eh0"][:], sT[:],
                             start=True, stop=True)
            nc.tensor.matmul(ps[:, 128:256], cs["Eh1"][:], sT[:],
                             start=True, stop=True)
            return sbuf_copy(ps, "bc_" + tag, bufs=2)

        m_eb = bcast_pk(stats4[:, 0:16], "me")
        rsd_eb = bcast_pk(stats4[:, 16:32], "re")
        m_tb = bcast_pk(stats4[:, 32:48], "mt")
        rsd_tb = bcast_pk(stats4[:, 48:64], "rt")

        z_e = mid("z_e")
        nc.vector.tensor_sub(z_e[:], avg_pk[:], m_eb[:])
        nc.vector.tensor_mul(z_e[:], z_e[:], rsd_eb[:])
        z_t = mid("z_t")
        nc.vector.tensor_sub(z_t[:], avg_pk[:], m_tb[:])
        nc.vector.tensor_mul(z_t[:], z_t[:], rsd_tb[:])

        # ===== packed softmax (no max-shift; z is standardized) =====
        ex_e = mid("ex_e")
        nc.scalar.activation(ex_e[:], z_e[:], AF.Exp)
        ex_t = mid("ex_t")
        nc.scalar.activation(ex_t[:], z_t[:], AF.Exp)

        def softmax_tail(z, ex, sfx):
            esp = mid("esp" + sfx, [128, 1])
            nc.vector.reduce_sum(esp[:],
                                 ex.rearrange("p (o j) -> p o j", o=1)[:],
                                 axis=AX.X)
            esbc = ee_bcast(esp)
            ln128 = mid("ln128" + sfx, [128, 1])
            nc.scalar.activation(ln128[:], esbc[:], AF.Ln)
            rs128 = mid("rs128" + sfx, [128, 1])
            nc.vector.reciprocal(rs128[:], esbc[:])
            lsf = mid("lsf" + sfx)
            nc.vector.tensor_scalar(lsf[:], z[:], ln128[:], None, AL.subtract)
            p = mid("p" + sfx)
            nc.vector.tensor_scalar(p[:], ex[:], rs128[:], None, AL.mult)
            return lsf, p

        lsf_e, p_e = softmax_tail(z_e, ex_e, "e")
        lsf_t, p_t = softmax_tail(z_t, ex_t, "t")

        diff = mid("diff")
        nc.vector.tensor_sub(diff[:], lsf_e[:], lsf_t[:])
        pd = mid("pd")
        nc.vector.tensor_mul(pd[:], p_e[:], diff[:])
        kp = mid("kp", [128, 1])
        nc.vector.reduce_sum(kp[:], pd.rearrange("p (o j) -> p o j", o=1)[:],
                             axis=AX.X)
        klbc = ee_bcast(kp)
        G_env = mid("G_env")
        nc.vector.tensor_scalar(G_env[:], diff[:], klbc[:], None, AL.subtract)
        nc.vector.tensor_mul(G_env[:], p_e[:], G_env[:])
        nc.vector.tensor_scalar(G_env[:], G_env[:], 0.0078125, None, AL.mult)
        G_tot = mid("G_tot")
        nc.vector.tensor_sub(G_tot[:], p_t[:], p_e[:])
        nc.vector.tensor_scalar(G_tot[:], G_tot[:], 0.0078125, None, AL.mult)
        g_ve = mid("g_ve")
        nc.vector.tensor_mul(g_ve[:], G_env[:], z_e[:])
        g_vt = mid("g_vt")
        nc.vector.tensor_mul(g_vt[:], G_tot[:], z_t[:])

        def pert_scale(g, sfx):
            """rho / sqrt(||g||_row + 1e-12), replicated [128,1]."""
            sq = mid("psq", [128, 256])
            nc.vector.tensor_mul(sq[:], g[:], g[:])
            np_ = mid("pnp" + sfx, [128, 1])
            nc.vector.reduce_sum(np_[:],
                                 sq.rearrange("p (o j) -> p o j", o=1)[:],
                                 axis=AX.X)
            bc = ee_bcast(np_)
            s = mid("ps" + sfx, [128, 1])
            nc.scalar.activation(s[:], bc[:], AF.Sqrt)
            nc.vector.tensor_scalar(s[:], s[:], 1e-12, None, AL.add)
            nc.scalar.activation(s[:], s[:], AF.Sqrt)
            nc.vector.reciprocal(s[:], s[:])
            nc.vector.tensor_scalar(s[:], s[:], RHO, None, AL.mult)
            return s

        s_me = pert_scale(G_env, "a")
        s_ve = pert_scale(g_ve, "b")
        s_mt = pert_scale(G_tot, "c")
        s_vt = pert_scale(g_vt, "d")

        def align2(z, gm, gv, s_m, s_v, sfx):
            d = mid("d" + sfx)
            nc.vector.tensor_scalar(d[:], gv[:], s_v[:], None, AL.mult)
            nc.vector.tensor_scalar(d[:], d[:], 1.0, None, AL.add)
            out = mid("a2" + sfx)
            nc.vector.tensor_mul(out[:], z[:], d[:])
            dm = mid("dm" + sfx)
            nc.vector.tensor_scalar(dm[:], gm[:], s_m[:], None, AL.mult)
            nc.vector.tensor_add(out[:], out[:], dm[:])
            return out

        env_a2 = align2(z_e, G_env, g_ve, s_me, s_ve, "e")
        tot_a2 = align2(z_t, G_tot, g_vt, s_mt, s_vt, "t")

        # =========== gram (psum over batch) ===========
        w1 = mid("w1")
        nc.vector.tensor_scalar(w1[:], tot_a2[:], 1e-07, None, AL.add)
        w2 = mid("w2")
        nc.vector.tensor_scalar(w2[:], env_a2[:], 1e-07, None, AL.add)
        nc.vector.tensor_mul(w1[:], w1[:], w2[:])
        gps = psB([8, 256])
        nc.tensor.matmul(gps[:], cs["E8s"][:], w1[:], start=True, stop=True)
        gsb = sbuf_copy(gps, "gsb")
        gram_i = dram.tile([8, 256], f32, tag="gram_i")
        nc.sync.dma_start(gram_i[:], gsb[:])
        gram_o = dram.tile([NCORES, 8, 256], f32, tag="gram_o")
        nc.gpsimd.collective_compute(
            "AllGather", AL.bypass, replica_groups=[list(range(NCORES))],
            ins=[gram_i.opt()], outs=[gram_o.opt()])
        gram64 = pool.tile([NCORES * 8, 256], f32, tag="gram64")
        nc.sync.dma_start(gram64[:],
                          gram_o.rearrange("r p j -> (r p) j")[:])
        g8ps = psB([8, 256])
        nc.tensor.matmul(g8ps[:], cs["E8s"][0:64, :], gram64[:], start=True,
                         stop=True)
        rg8 = pool.tile([8, 256], f32, tag="rg8")
        nc.vector.reciprocal(rg8[:], g8ps[:])
        rps = psB([128, 256])
        nc.tensor.matmul(rps[:], cs["E8b"][:], rg8[:], start=True, stop=True)
        rgramb = sbuf_copy(rps, "rgramb")

        t3 = mid("t3")
        nc.vector.tensor_mul(t3[:], tot_a2[:], rgramb[:])
        e3 = mid("e3")
        nc.vector.tensor_mul(e3[:], env_a2[:], rgramb[:])

        # =========== minmax -> scores -> inv_s ===========
        def mm_c2(v, sfx):
            """row extremes of packed v -> [16,2] sbuf (col0=max, col1=-min)."""
            vneg = mid("vneg", [128, 256])
            nc.vector.tensor_scalar(vneg[:], v[:], -1.0, None, AL.mult)
            mm2 = mid("mm2" + sfx, [128, 2])
            nc.vector.tensor_reduce(mm2[:, 0:1], v[:], axis=AX.X, op=AL.max)
            nc.vector.tensor_reduce(mm2[:, 1:2], vneg[:], axis=AX.X, op=AL.max)
            tp = psB([2, 128])
            nc.tensor.transpose(tp[:], mm2[:], ident[:])
            s2 = sbuf_copy(tp, "s2" + sfx, bufs=2)
            red2 = pool.tile([2, 16], f32, tag="red2", bufs=2,
                             name=f"red2_{next(_n)}")
            nc.vector.tensor_reduce(red2[:],
                                    s2.rearrange("t (b q) -> t b q", q=8)[:],
                                    axis=AX.X, op=AL.max)
            tp2 = psB([16, 2])
            nc.tensor.transpose(tp2[:], red2[:], ident[0:2, 0:2])
            return sbuf_copy(tp2, "c2" + sfx, bufs=2)

        def minmax_pk(v, sfx):
            c2 = mm_c2(v, sfx)
            den16 = mid("den16" + sfx, [16, 1])
            nc.vector.tensor_add(den16[:], c2[:, 0:1], c2[:, 1:2])
            nc.vector.reciprocal(den16[:], den16[:])
            mnbc = psC([128, 1])   # broadcast of -min
            nc.tensor.matmul(mnbc[:], cs["E16b"][:], c2[:, 1:2], start=True,
                             stop=True)
            rbc = psC([128, 1])
            nc.tensor.matmul(rbc[:], cs["E16b"][:], den16[:], start=True,
                             stop=True)
            num = mid("num" + sfx)
            nc.vector.tensor_scalar(num[:], v[:], mnbc[:], None, AL.add)
            nc.vector.tensor_scalar(num[:], num[:], rbc[:], None, AL.mult)
            return num

        t4 = minmax_pk(t3, "t")
        e4 = minmax_pk(e3, "e")
        sqd = mid("sqd")
        nc.vector.tensor_sub(sqd[:], t4[:], e4[:])
        nc.vector.tensor_mul(sqd[:], sqd[:], sqd[:])

        # inv_s = (rowmax-rowmin) / (sqd - rowmin)
        c2s = mm_c2(sqd, "s")
        num16 = mid("num16", [16, 1])
        nc.vector.tensor_add(num16[:], c2s[:, 0:1], c2s[:, 1:2])
        numbc = psC([128, 1])
        nc.tensor.matmul(numbc[:], cs["E16b"][:], num16[:], start=True,
                         stop=True)
        mnbc2 = psC([128, 1])   # broadcast of -min
        nc.tensor.matmul(mnbc2[:], cs["E16b"][:], c2s[:, 1:2], start=True,
                         stop=True)
        den2 = mid("den2")
        nc.vector.tensor_scalar(den2[:], sqd[:], mnbc2[:], None, AL.add)
        nc.vector.reciprocal(den2[:], den2[:])
        inv_s = mid("inv_s")
        nc.vector.tensor_scalar(inv_s[:], den2[:], numbc[:], None, AL.mult)

        g = mid("g")
        nc.vector.tensor_mul(g[:], lnr[:], inv_s[:])

        # ===== multi-probe search, state replicated per partition =====
        lo = pool.tile([128, 1], f32, tag="s_lo", bufs=2)
        nc.gpsimd.memset(lo[:], LO0)
        hi = pool.tile([128, 1], f32, tag="s_hi", bufs=2)
        nc.gpsimd.memset(hi[:], 0.0)
        cjunk = mid("cjunk")
        for it in range(SEARCH_ROUNDS):
            w8 = pool.tile([128, 1], f32, tag="s_w8", name=f"w8_{next(_n)}",
                           bufs=2)
            nc.vector.tensor_sub(w8[:], hi[:], lo[:])
            nc.vector.tensor_scalar(w8[:], w8[:], 0.125, None, AL.mult)
            mids = pool.tile([128, NMID], f32, tag="s_mid",
                             name=f"mids_{next(_n)}", bufs=2)
            nc.vector.tensor_scalar(mids[:], cs["K7r"][:, 0:NMID], w8[:], None,
                                    AL.mult)
            nc.vector.tensor_scalar(mids[:], mids[:], lo[:], None, AL.add)
            cnt7 = pool.tile([128, NMID], f32, tag="s_cnt7",
                             name=f"cnt7_{next(_n)}", bufs=2)
            for i in range(NMID):
                nc.vector.tensor_scalar(cjunk[:], g[:], mids[:, i:i + 1], None,
                                        AL.is_gt, op1=AL.add,
                                        accum_out=cnt7[:, i:i + 1])
            cps = psB([128, NMID])
            nc.tensor.matmul(cps[:], cs["EE"][:], cnt7[:], start=True,
                             stop=True)
            flags = pool.tile([128, NMID], f32, tag="s_flag",
                              name=f"flag_{next(_n)}", bufs=2)
            nc.vector.tensor_scalar(flags[:], cps[:], KF, None, AL.is_gt)
            s16 = pool.tile([128, 1], f32, tag="s_s16", name=f"s16_{next(_n)}",
                            bufs=2)
            nc.vector.reduce_sum(s16[:],
                                 flags.rearrange("p (o j) -> p o j", o=1)[:],
                                 axis=AX.X)
            step = pool.tile([128, 1], f32, tag="s_step",
                             name=f"step_{next(_n)}", bufs=2)
            nc.vector.tensor_mul(step[:], s16[:], w8[:])
            lo2 = pool.tile([128, 1], f32, tag="s_lo", name=f"lo_{next(_n)}",
                            bufs=2)
            nc.vector.tensor_add(lo2[:], lo[:], step[:])
            hi2 = pool.tile([128, 1], f32, tag="s_hi", name=f"hi_{next(_n)}",
                            bufs=2)
            nc.vector.tensor_add(hi2[:], lo2[:], w8[:])
            lo, hi = lo2, hi2

        # global masked-out count -> kick collective early (overlaps thr/mask)
        cnt_f = mid("cnt_f", [128, 1])
        nc.vector.tensor_scalar(cjunk[:], g[:], hi[:], None, AL.is_gt,
                                op1=AL.add, accum_out=cnt_f[:])
        totp = psC([1, 1])
        nc.tensor.matmul(totp[:], cs["ones128"][:], cnt_f[:], start=True,
                         stop=True)
        tot_above = sbuf_copy(totp, "tot_above")
        cnt_i = dram.tile([1, 1], f32, tag="cnt_i")
        nc.sync.dma_start(cnt_i[:], tot_above[:])
        cnt_o = dram.tile([NCORES, 1], f32, tag="cnt_o")
        nc.gpsimd.collective_compute(
            "AllGather", AL.bypass, replica_groups=[list(range(NCORES))],
            ins=[cnt_i.opt()], outs=[cnt_o.opt()])

        # thr = rowmax(g where g <= hi); hi already replicated per partition
        selm = mid("selm", [128, 256], u8)
        nc.vector.tensor_scalar(selm[:], g[:], hi[:], None, AL.is_le)
        gm = mid("gmz")
        nc.gpsimd.memset(gm[:], -1.0e38)
        nc.vector.copy_predicated(gm[:], selm[:], g[:])
        gmx = mid("gmx", [128, 1])
        nc.vector.tensor_reduce(gmx[:], gm[:], axis=AX.X, op=AL.max)
        tpx = psB([1, 128])
        nc.tensor.transpose(tpx[:], gmx[:], ident[:])
        sx = sbuf_copy(tpx, "sx")
        redx = pool.tile([1, 16], f32, tag="redx")
        nc.vector.tensor_reduce(redx[:],
                                sx.rearrange("o (b q) -> o b q", q=8)[:],
                                axis=AX.X, op=AL.max)
        tpx2 = psB([16, 1])
        nc.tensor.transpose(tpx2[:], redx[:], ident[0:1, 0:1])
        thr16 = sbuf_copy(tpx2, "thr16")
        thrbc = psC([128, 1])
        nc.tensor.matmul(thrbc[:], cs["E16b"][:], thr16[:], start=True,
                         stop=True)
        mask01 = mid("mask01")
        nc.vector.tensor_scalar(mask01[:], g[:], thrbc[:], None, AL.is_le)

        # scale = 262144 / (262144 - total_above)
        allc = pool.tile([1, NCORES], f32, tag="allc")
        nc.sync.dma_start(allc[:], cnt_o.rearrange("r o -> o r")[:])
        tota = pool.tile([1, 1], f32, tag="tota")
        nc.vector.reduce_sum(tota[:],
                             allc.rearrange("o (a r) -> o a r", a=1)[:],
                             axis=AX.X)
        scl = pool.tile([1, 1], f32, tag="scl")
        nc.vector.tensor_scalar(scl[:], tota[:], -1.0, 262144.0, AL.mult,
                                op1=AL.add)
        nc.vector.reciprocal(scl[:], scl[:])
        nc.vector.tensor_scalar(scl[:], scl[:], 262144.0, None, AL.mult)
        sclbc = psC([128, 1])
        nc.tensor.matmul(sclbc[:], cs["ones1"][:], scl[:], start=True, stop=True)
        maskS = mid("maskS")
        nc.vector.tensor_scalar(maskS[:], mask01[:], sclbc[:], None, AL.mult)

        # mask columns: smT_h[p, b*8+q] = scaled mask at c' = q*256+h*128+p
        smt_list = []
        for h in range(2):
            tph = psB([128, 128])
            nc.tensor.transpose(tph[:], maskS[:, h * 128:(h + 1) * 128],
                                ident[:])
            sm = pool.tile([128, 128], f32, tag=f"smT{h}")
            nc.scalar.copy(sm[:], tph[:])
            smt_list.append(sm)

        # =========== PHASE C: mask-multiply from cache, store ===========
        ov = out_d.rearrange("b (p jj) h -> b p (jj h)", p=128)
        nhalf = 0
        for b in range(BL):
            for h in range(2):
                ot = xpool.tile([128, HFREE], f32, tag="xa")
                for jh in range(NJ // 2):
                    jj = h * 8 + jh
                    col = b * 8 + jj // 2
                    smcol = smt_list[jj % 2][:, col:col + 1]
                    src = cache[:,
                                b * FREE_B + jj * HW:b * FREE_B + (jj + 1) * HW]
                    dst = ot[:, jh * HW:(jh + 1) * HW]
                    if jj % 4 == 3:
                        nc.scalar.activation(dst, src, AF.Copy, scale=smcol)
                    else:
                        nc.vector.tensor_scalar(dst, src, smcol, None, AL.mult)
                dstv = ov[b, :, h * HFREE:(h + 1) * HFREE]
                if nhalf % 2 == 0:
                    nc.sync.dma_start(dstv, ot[:])
                else:
                    nc.gpsimd.dma_start(dstv, ot[:])
                nhalf += 1

    nc.finalize()
    return nc


def kernel(x, r, ratio, rho):
    x = np.ascontiguousarray(np.asarray(x, dtype=np.float32))
    r = np.ascontiguousarray(np.asarray(r, dtype=np.float32))
    ratio_f = float(np.asarray(ratio))
    rho_f = float(np.asarray(rho))
    k = int(ratio_f * C)
    key = (k, np.float32(rho_f).tobytes())
    if key not in _CACHE:
        _CACHE[key] = build(k, rho_f)
    nc = _CACHE[key]

    consts = _consts()
    xr = x.reshape(B, C, HW)
    in_maps = []
    for c in range(NCORES):
        envm = np.zeros((128, NCORES * 32), np.float32)
        e = c // 2
        envm[:, (2 * e) * 32:(2 * e + 2) * 32] = 1.0
        m = {"x": np.ascontiguousarray(xr[c * BL:(c + 1) * BL]),
             "r": np.ascontiguousarray(r[c * BL:(c + 1) * BL]),
             "envm": envm}
        m.update(consts)
        in_maps.append(m)
    res = run_bass_kernel_spmd(nc, in_maps, core_ids=list(range(NCORES)),
                               tmpdir=os.environ.get("BASS_TMPDIR"))
    LAST["res"] = res
    out = np.concatenate([res.results[c]["out"].reshape(BL, C, HW)
                          for c in range(NCORES)], axis=0)
    return out.reshape(B, C, 14, 14)


# revision 37
# speedup vs baseline: 1.1213x; 1.1213x over previous
"""DgCD forward (topk channel masking) on 8 Trainium2 NeuronCores.

v3: fully sharded middle + SBUF-cached x + fused combine-broadcast matmuls.
  - Phase A: per-row channel-block loads ([128p, 16*196], contiguous 12.5KB
    lines), avg-pool reduce, bf16 x-cache in SBUF.
  - Middle: each core computes scores/top-k only for its own 16 batch rows in
    the packed [(16b x 8q), 256] layout; cross-core coupling via 4 small
    collectives (env-pair sums, total sums, gram diag, mask count).
  - Row-wide sums use one EE=E16c@E16b matmul (combine+broadcast fused); the
    top-k search keeps lo/hi replicated per partition so each round is one
    matmul plus vector work. Softmax skips max-subtraction (z standardized).
  - Phase C: mask-multiply from the bf16 cache, store only (no x re-read).
Channel order in the middle is block-permuted (c' = (c%16)*128 + c//16); all
middle math is channel-permutation-equivariant and phase C maps the mask back.
"""
import os
import sys
sys.path.insert(0, "/opt/trn_rl_repo")
import numpy as np
from contextlib import ExitStack

import concourse.bass as bass
import concourse.bacc as bacc_mod
import concourse.mybir as mybir
import concourse.tile as tile
from concourse.bass_utils import run_bass_kernel_spmd

f32 = mybir.dt.float32
bf16 = mybir.dt.bfloat16
u8 = mybir.dt.uint8
AL = mybir.AluOpType
AF = mybir.ActivationFunctionType
AX = mybir.AxisListType

B, C, HW = 128, 2048, 196
NCORES = 8
BL = B // NCORES          # 16 batch rows per core
NJ = 16                   # sub-channels per partition block
NQ = 8                    # 256-wide packed chunks per row
FREE_B = NJ * HW          # 3136 floats per partition per row
NMID = 7                  # thresholds probed per search round
SEARCH_ROUNDS = 6         # 8^6 = 2^18 bracket shrink
LO0 = -104.0

C196 = float(np.float32(1.0 / 196.0))
C31 = float(np.float32(1.0 / 31.0))
C127 = float(np.float32(1.0 / 127.0))

_CACHE = {}
LAST = {}


def _consts():
    ident = np.eye(128, dtype=np.float32)
    E16b = np.zeros((16, 128), np.float32)   # [16,1] row vals -> [128,1] bcast
    for p in range(128):
        E16b[p // 8, p] = 1.0
    EE = np.zeros((128, 128), np.float32)    # row-sum + bcast in one matmul
    for kk in range(128):
        for p in range(128):
            if kk // 8 == p // 8:
                EE[kk, p] = 1.0
    Eh0 = np.zeros((16, 128), np.float32)    # statT [16,128] -> packed halves
    Eh1 = np.zeros((16, 128), np.float32)
    for p in range(128):
        Eh0[2 * (p % 8), p] = 1.0
        Eh1[2 * (p % 8) + 1, p] = 1.0
    E8s = np.zeros((128, 8), np.float32)     # sum over b for fixed q
    E8b = np.zeros((8, 128), np.float32)     # [8,256] chunk stats -> [128,256]
    for p in range(128):
        E8s[p, p % 8] = 1.0
        E8b[p % 8, p] = 1.0
    ones1 = np.ones((1, 128), np.float32)
    ones128 = np.ones((128, 1), np.float32)
    K7r = np.zeros((128, NMID), np.float32)
    for i in range(NMID):
        K7r[:, i] = float(i + 1)
    return {"ident": ident, "E16b": E16b, "EE": EE, "Eh0": Eh0, "Eh1": Eh1,
            "E8s": E8s, "E8b": E8b, "ones1": ones1, "ones128": ones128,
            "K7r": K7r}


def build(k, rho):
    nc = bacc_mod.Bacc()
    x_d = nc.dram_tensor("x", [BL, C, HW], f32, kind="ExternalInput")
    r_d = nc.dram_tensor("r", [BL, C], f32, kind="ExternalInput")
    envm_d = nc.dram_tensor("envm", [128, 256], f32, kind="ExternalInput")
    cd = {n: nc.dram_tensor(n, list(v.shape), f32, kind="ExternalInput")
          for n, v in _consts().items()}
    out_d = nc.dram_tensor("out", [BL, C, HW], f32, kind="ExternalOutput")

    RHO = float(np.float32(rho))
    KF = float(k)

    with tile.TileContext(nc) as tc, ExitStack() as ctx:
        pool = ctx.enter_context(tc.tile_pool(name="main", bufs=1))
        big = ctx.enter_context(tc.tile_pool(name="bigp", bufs=1))
        psum = ctx.enter_context(tc.tile_pool(name="psum", bufs=1, space="PSUM"))
        dram = ctx.enter_context(tc.tile_pool(name="dram", bufs=1, space="DRAM"))
        xpool = ctx.enter_context(tc.tile_pool(name="xio", bufs=4))

        _n = iter(range(100000))

        def psB(shape):
            return psum.tile(shape, f32, tag="psB", bufs=3,
                             name=f"psB_{next(_n)}", padded_shape=[128, 256])

        def psC(shape):
            return psum.tile(shape, f32, tag="psC", bufs=4,
                             name=f"psC_{next(_n)}", padded_shape=[128, 1])

        def mid(tag, shape=None, dt=f32):
            return pool.tile(shape or [128, 256], dt, tag=tag,
                             name=f"{tag}_{next(_n)}")

        # ---- constants ----
        cs = {}
        for n, v in _consts().items():
            cs[n] = pool.tile(list(v.shape), f32, tag="c_" + n, name="c_" + n)
            nc.gpsimd.dma_start(cs[n][:], cd[n][:])
        ident = cs["ident"]

        def sbuf_copy(ps, tag, bufs=1):
            t = pool.tile([ps.shape[0], ps.shape[1]], f32, tag=tag,
                          name=f"sc_{tag}_{next(_n)}", bufs=bufs)
            nc.scalar.copy(t[:], ps[:])
            return t

        def ee_bcast(val_pp):
            """[128,1] per-partition -> per-row sums broadcast [128,1] PSUM."""
            ps = psC([128, 1])
            nc.tensor.matmul(ps[:], cs["EE"][:], val_pp[:], start=True,
                             stop=True)
            return ps

        # =========== PHASE A: load x, avg-pool, cache bf16 ===========
        r_s = pool.tile([BL, C], f32, tag="rp16", bufs=2, name="r_s")
        nc.sync.dma_start(r_s[:], r_d[:])

        cache = big.tile([128, BL * FREE_B], bf16, tag="xcache")
        avgw = pool.tile([128, NJ * BL], f32, tag="avgw")   # free = (jj, b)
        HFREE = FREE_B // 2
        xv = x_d.rearrange("b (p jj) h -> b p (jj h)", p=128)
        nhalf = 0
        for b in range(BL):
            for h in range(2):
                xt = xpool.tile([128, HFREE], f32, tag="xa")
                src = xv[b, :, h * HFREE:(h + 1) * HFREE]
                if nhalf % 2 == 0:
                    nc.sync.dma_start(xt[:], src)
                else:
                    nc.gpsimd.dma_start(xt[:], src)
                nhalf += 1
                nc.vector.reduce_sum(
                    avgw.rearrange("p (jj b) -> p jj b", b=BL)
                    [:, h * 8:(h + 1) * 8, b],
                    xt.rearrange("p (jj hh) -> p jj hh", jj=NJ // 2)[:],
                    axis=AX.X)
                nc.scalar.activation(
                    cache[:, b * FREE_B + h * HFREE:b * FREE_B + (h + 1) * HFREE],
                    xt[:], AF.Copy)

        envm = pool.tile([128, 256], f32, tag="envm")
        nc.sync.dma_start(envm[:], envm_d[:])

        # ---- per-channel batch sums -> AllGather (faster than AllReduce) ----
        nc.vector.tensor_scalar(avgw[:], avgw[:], C196, None, AL.mult)
        sqw = pool.tile([128, NJ * BL], f32, tag="sqw")
        nc.vector.tensor_mul(sqw[:], avgw[:], avgw[:])
        st = pool.tile([128, 32], f32, tag="st")
        nc.vector.reduce_sum(st[:, 0:16],
                             avgw.rearrange("p (jj b) -> p jj b", b=BL)[:],
                             axis=AX.X)
        nc.vector.reduce_sum(st[:, 16:32],
                             sqw.rearrange("p (jj b) -> p jj b", b=BL)[:],
                             axis=AX.X)
        st_in = dram.tile([128, 32], f32, tag="st_in")
        nc.sync.dma_start(st_in[:], st[:])
        st_out_d = dram.tile([NCORES, 128, 32], f32, tag="st_out_d")
        nc.gpsimd.collective_compute(
            "AllGather", AL.bypass, replica_groups=[list(range(NCORES))],
            ins=[st_in.opt()], outs=[st_out_d.opt()])

        # ---- r -> permuted packed + ln(r)  (overlaps collectives) ----
        r_rp = pool.tile([BL, C], f32, tag="rp16", bufs=2, name="r_rp")
        nc.vector.tensor_copy(r_rp.rearrange("b (jj p) -> b jj p", jj=NJ)[:],
                              r_s.rearrange("b (p jj) -> b jj p", jj=NJ)[:])
        r_rt = dram.tile([BL, C], f32, tag="r_rt")
        nc.sync.dma_start(r_rt[:], r_rp[:])
        r_pk = mid("r_pk")
        nc.sync.dma_start(r_pk[:], r_rt.rearrange("b (q j) -> (b q) j", q=NQ)[:])
        lnr = mid("lnr")
        nc.scalar.activation(lnr[:], r_pk[:], AF.Ln)

        # ---- avg -> row-permuted -> packed  (overlaps collectives) ----
        avg_rp = pool.tile([BL, C], f32, tag="rp16", bufs=2, name="avg_rp")
        for jj in range(NJ):
            tp = psB([BL, 128])
            nc.tensor.transpose(tp[:], avgw[:, jj * BL:(jj + 1) * BL], ident[:])
            nc.vector.tensor_copy(avg_rp[:, jj * 128:(jj + 1) * 128], tp[:])
        avg_rt = dram.tile([BL, C], f32, tag="avg_rt")
        nc.sync.dma_start(avg_rt[:], avg_rp[:])
        avg_pk = mid("avg_pk")
        nc.sync.dma_start(avg_pk[:],
                          avg_rt.rearrange("b (q j) -> (b q) j", q=NQ)[:])

        # =========== stats -> z ===========
        st_all = pool.tile([128, NCORES * 32], f32, tag="st_all")
        nc.sync.dma_start(st_all.rearrange("p (r s) -> p r s", r=NCORES)[:],
                          st_out_d.rearrange("r p s -> p r s")[:])
        st_tot = pool.tile([128, 32], f32, tag="st_tot")
        nc.vector.reduce_sum(st_tot[:],
                             st_all.rearrange("p (r s) -> p s r", r=NCORES)[:],
                             axis=AX.X)
        envp = pool.tile([128, NCORES * 32], f32, tag="envp")
        nc.vector.tensor_mul(envp[:], st_all[:], envm[:])
        st_env = pool.tile([128, 32], f32, tag="st_env")
        nc.vector.reduce_sum(st_env[:],
                             envp.rearrange("p (r s) -> p s r", r=NCORES)[:],
                             axis=AX.X)

        stats4 = pool.tile([128, 64], f32, tag="stats4")  # m_e|rsd_e|m_t|rsd_t

        def mk_stats(src, dst_m, dst_r, n, cinv):
            nc.vector.tensor_scalar(dst_m, src[:, 0:16], 1.0 / n, None, AL.mult)
            t = mid("vtmp", [128, 16])
            nc.vector.tensor_mul(t[:], dst_m, dst_m)
            nc.vector.tensor_scalar(t[:], t[:], float(n), None, AL.mult)
            v = mid("vvar", [128, 16])
            nc.vector.tensor_sub(v[:], src[:, 16:32], t[:])
            nc.vector.tensor_scalar(v[:], v[:], cinv, 1e-05, AL.mult, op1=AL.add)
            sd = mid("vsd", [128, 16])
            nc.scalar.activation(sd[:], v[:], AF.Sqrt)
            nc.vector.reciprocal(dst_r, sd[:])

        mk_stats(st_env, stats4[:, 0:16], stats4[:, 16:32], 32, C31)
        mk_stats(st_tot, stats4[:, 32:48], stats4[:, 48:64], 128, C127)

        def bcast_pk(src16, tag):
            """[128,16] per-(p,jj) channel stat -> packed [128,256] bcast."""
            tp = psB([16, 128])
            nc.tensor.transpose(tp[:], src16, ident[:])
            sT = sbuf_copy(tp, "sT_" + tag)
            ps = psB([128, 256])
            nc.tensor.matmul(ps[:, 0:128], cs["Eh0"][:], sT[:],
                             start=True, stop=True)
            nc.tensor.matmul(ps[:, 128:256], cs["Eh1"][:], sT[:],
                             start=True, stop=True)
            return sbuf_copy(ps, "bc_" + tag, bufs=2)

        m_eb = bcast_pk(stats4[:, 0:16], "me")
        rsd_eb = bcast_pk(stats4[:, 16:32], "re")
        m_tb = bcast_pk(stats4[:, 32:48], "mt")
        rsd_tb = bcast_pk(stats4[:, 48:64], "rt")

        z_e = mid("z_e")
        nc.vector.tensor_sub(z_e[:], avg_pk[:], m_eb[:])
        nc.vector.tensor_mul(z_e[:], z_e[:], rsd_eb[:])
        z_t = mid("z_t")
        nc.vector.tensor_sub(z_t[:], avg_pk[:], m_tb[:])
        nc.vector.tensor_mul(z_t[:], z_t[:], rsd_tb[:])

        # ===== packed softmax (no max-shift; z is standardized) =====
        ex_e = mid("ex_e")
        nc.scalar.activation(ex_e[:], z_e[:], AF.Exp)
        ex_t = mid("ex_t")
        nc.scalar.activation(ex_t[:], z_t[:], AF.Exp)

        def softmax_tail(z, ex, sfx):
            esp = mid("esp" + sfx, [128, 1])
            nc.vector.reduce_sum(esp[:],
                                 ex.rearrange("p (o j) -> p o j", o=1)[:],
                                 axis=AX.X)
            esbc = ee_bcast(esp)
            ln128 = mid("ln128" + sfx, [128, 1])
            nc.scalar.activation(ln128[:], esbc[:], AF.Ln)
            rs128 = mid("rs128" + sfx, [128, 1])
            nc.vector.reciprocal(rs128[:], esbc[:])
            lsf = mid("lsf" + sfx)
            nc.vector.tensor_scalar(lsf[:], z[:], ln128[:], None, AL.subtract)
            p = mid("p" + sfx)
            nc.vector.tensor_scalar(p[:], ex[:], rs128[:], None, AL.mult)
            return lsf, p

        lsf_e, p_e = softmax_tail(z_e, ex_e, "e")
        lsf_t, p_t = softmax_tail(z_t, ex_t, "t")

        diff = mid("diff")
        nc.vector.tensor_sub(diff[:], lsf_e[:], lsf_t[:])
        pd = mid("pd")
        nc.vector.tensor_mul(pd[:], p_e[:], diff[:])
        kp = mid("kp", [128, 1])
        nc.vector.reduce_sum(kp[:], pd.rearrange("p (o j) -> p o j", o=1)[:],
                             axis=AX.X)
        klbc = ee_bcast(kp)
        G_env = mid("G_env")
        nc.vector.tensor_scalar(G_env[:], diff[:], klbc[:], None, AL.subtract)
        nc.vector.tensor_mul(G_env[:], p_e[:], G_env[:])
        nc.vector.tensor_scalar(G_env[:], G_env[:], 0.0078125, None, AL.mult)
        G_tot = mid("G_tot")
        nc.vector.tensor_sub(G_tot[:], p_t[:], p_e[:])
        nc.vector.tensor_scalar(G_tot[:], G_tot[:], 0.0078125, None, AL.mult)
        g_ve = mid("g_ve")
        nc.vector.tensor_mul(g_ve[:], G_env[:], z_e[:])
        g_vt = mid("g_vt")
        nc.vector.tensor_mul(g_vt[:], G_tot[:], z_t[:])

        def pert_scale(g, sfx):
            """rho / sqrt(||g||_row + 1e-12), replicated [128,1]."""
            sq = mid("psq", [128, 256])
            nc.vector.tensor_mul(sq[:], g[:], g[:])
            np_ = mid("pnp" + sfx, [128, 1])
            nc.vector.reduce_sum(np_[:],
                                 sq.rearrange("p (o j) -> p o j", o=1)[:],
                                 axis=AX.X)
            bc = ee_bcast(np_)
            s = mid("ps" + sfx, [128, 1])
            nc.scalar.activation(s[:], bc[:], AF.Sqrt)
            nc.vector.tensor_scalar(s[:], s[:], 1e-12, None, AL.add)
            nc.scalar.activation(s[:], s[:], AF.Sqrt)
            nc.vector.reciprocal(s[:], s[:])
            nc.vector.tensor_scalar(s[:], s[:], RHO, None, AL.mult)
            return s

        s_me = pert_scale(G_env, "a")
        s_ve = pert_scale(g_ve, "b")
        s_mt = pert_scale(G_tot, "c")
        s_vt = pert_scale(g_vt, "d")

        def align2(z, gm, gv, s_m, s_v, sfx):
            d = mid("d" + sfx)
            nc.vector.tensor_scalar(d[:], gv[:], s_v[:], None, AL.mult)
            nc.vector.tensor_scalar(d[:], d[:], 1.0, None, AL.add)
            out = mid("a2" + sfx)
            nc.vector.tensor_mul(out[:], z[:], d[:])
            dm = mid("dm" + sfx)
            nc.vector.tensor_scalar(dm[:], gm[:], s_m[:], None, AL.mult)
            nc.vector.tensor_add(out[:], out[:], dm[:])
            return out

        env_a2 = align2(z_e, G_env, g_ve, s_me, s_ve, "e")
        tot_a2 = align2(z_t, G_tot, g_vt, s_mt, s_vt, "t")

        # =========== gram (psum over batch) ===========
        w1 = mid("w1")
        nc.vector.tensor_scalar(w1[:], tot_a2[:], 1e-07, None, AL.add)
        w2 = mid("w2")
        nc.vector.tensor_scalar(w2[:], env_a2[:], 1e-07, None, AL.add)
        nc.vector.tensor_mul(w1[:], w1[:], w2[:])
        gps = psB([8, 256])
        nc.tensor.matmul(gps[:], cs["E8s"][:], w1[:], start=True, stop=True)
        gsb = sbuf_copy(gps, "gsb")
        gram_i = dram.tile([8, 256], f32, tag="gram_i")
        nc.sync.dma_start(gram_i[:], gsb[:])
        gram_o = dram.tile([NCORES, 8, 256], f32, tag="gram_o")
        nc.gpsimd.collective_compute(
            "AllGather", AL.bypass, replica_groups=[list(range(NCORES))],
            ins=[gram_i.opt()], outs=[gram_o.opt()])
        gram64 = pool.tile([NCORES * 8, 256], f32, tag="gram64")
        nc.sync.dma_start(gram64[:],
                          gram_o.rearrange("r p j -> (r p) j")[:])
        g8ps = psB([8, 256])
        nc.tensor.matmul(g8ps[:], cs["E8s"][0:64, :], gram64[:], start=True,
                         stop=True)
        rg8 = pool.tile([8, 256], f32, tag="rg8")
        nc.vector.reciprocal(rg8[:], g8ps[:])
        rps = psB([128, 256])
        nc.tensor.matmul(rps[:], cs["E8b"][:], rg8[:], start=True, stop=True)
        rgramb = sbuf_copy(rps, "rgramb")

        t3 = mid("t3")
        nc.vector.tensor_mul(t3[:], tot_a2[:], rgramb[:])
        e3 = mid("e3")
        nc.vector.tensor_mul(e3[:], env_a2[:], rgramb[:])

        # =========== minmax -> scores -> inv_s ===========
        def mm_c2(v, sfx):
            """row extremes of packed v -> [16,2] sbuf (col0=max, col1=-min)."""
            vneg = mid("vneg", [128, 256])
            nc.vector.tensor_scalar(vneg[:], v[:], -1.0, None, AL.mult)
            mm2 = mid("mm2" + sfx, [128, 2])
            nc.vector.tensor_reduce(mm2[:, 0:1], v[:], axis=AX.X, op=AL.max)
            nc.vector.tensor_reduce(mm2[:, 1:2], vneg[:], axis=AX.X, op=AL.max)
            tp = psB([2, 128])
            nc.tensor.transpose(tp[:], mm2[:], ident[:])
            s2 = sbuf_copy(tp, "s2" + sfx, bufs=2)
            red2 = pool.tile([2, 16], f32, tag="red2", bufs=2,
                             name=f"red2_{next(_n)}")
            nc.vector.tensor_reduce(red2[:],
                                    s2.rearrange("t (b q) -> t b q", q=8)[:],
                                    axis=AX.X, op=AL.max)
            tp2 = psB([16, 2])
            nc.tensor.transpose(tp2[:], red2[:], ident[0:2, 0:2])
            return sbuf_copy(tp2, "c2" + sfx, bufs=2)

        def minmax_pk(v, sfx):
            c2 = mm_c2(v, sfx)
            den16 = mid("den16" + sfx, [16, 1])
            nc.vector.tensor_add(den16[:], c2[:, 0:1], c2[:, 1:2])
            nc.vector.reciprocal(den16[:], den16[:])
            mnbc = psC([128, 1])   # broadcast of -min
            nc.tensor.matmul(mnbc[:], cs["E16b"][:], c2[:, 1:2], start=True,
                             stop=True)
            rbc = psC([128, 1])
            nc.tensor.matmul(rbc[:], cs["E16b"][:], den16[:], start=True,
                             stop=True)
            num = mid("num" + sfx)
            nc.vector.tensor_scalar(num[:], v[:], mnbc[:], None, AL.add)
            nc.vector.tensor_scalar(num[:], num[:], rbc[:], None, AL.mult)
            return num

        t4 = minmax_pk(t3, "t")
        e4 = minmax_pk(e3, "e")
        sqd = mid("sqd")
        nc.vector.tensor_sub(sqd[:], t4[:], e4[:])
        nc.vector.tensor_mul(sqd[:], sqd[:], sqd[:])

        # inv_s = (rowmax-rowmin) / (sqd - rowmin)
        c2s = mm_c2(sqd, "s")
        num16 = mid("num16", [16, 1])
        nc.vector.tensor_add(num16[:], c2s[:, 0:1], c2s[:, 1:2])
        numbc = psC([128, 1])
        nc.tensor.matmul(numbc[:], cs["E16b"][:], num16[:], start=True,
                         stop=True)
        mnbc2 = psC([128, 1])   # broadcast of -min
        nc.tensor.matmul(mnbc2[:], cs["E16b"][:], c2s[:, 1:2], start=True,
                         stop=True)
        den2 = mid("den2")
        nc.vector.tensor_scalar(den2[:], sqd[:], mnbc2[:], None, AL.add)
        nc.vector.reciprocal(den2[:], den2[:])
        inv_s = mid("inv_s")
        nc.vector.tensor_scalar(inv_s[:], den2[:], numbc[:], None, AL.mult)

        g = mid("g")
        nc.vector.tensor_mul(g[:], lnr[:], inv_s[:])

        # ===== multi-probe search, state replicated per partition =====
        lo = pool.tile([128, 1], f32, tag="s_lo", bufs=2)
        nc.gpsimd.memset(lo[:], LO0)
        hi = pool.tile([128, 1], f32, tag="s_hi", bufs=2)
        nc.gpsimd.memset(hi[:], 0.0)
        cjunk = mid("cjunk")
        for it in range(SEARCH_ROUNDS):
            w8 = pool.tile([128, 1], f32, tag="s_w8", name=f"w8_{next(_n)}",
                           bufs=2)
            nc.vector.tensor_sub(w8[:], hi[:], lo[:])
            nc.vector.tensor_scalar(w8[:], w8[:], 0.125, None, AL.mult)
            mids = pool.tile([128, NMID], f32, tag="s_mid",
                             name=f"mids_{next(_n)}", bufs=2)
            nc.vector.tensor_scalar(mids[:], cs["K7r"][:, 0:NMID], w8[:], None,
                                    AL.mult)
            nc.vector.tensor_scalar(mids[:], mids[:], lo[:], None, AL.add)
            cnt7 = pool.tile([128, NMID], f32, tag="s_cnt7",
                             name=f"cnt7_{next(_n)}", bufs=2)
            for i in range(NMID):
                nc.vector.tensor_scalar(cjunk[:], g[:], mids[:, i:i + 1], None,
                                        AL.is_gt, op1=AL.add,
                                        accum_out=cnt7[:, i:i + 1])
            cps = psB([128, NMID])
            nc.tensor.matmul(cps[:], cs["EE"][:], cnt7[:], start=True,
                             stop=True)
            flags = pool.tile([128, NMID], f32, tag="s_flag",
                              name=f"flag_{next(_n)}", bufs=2)
            nc.vector.tensor_scalar(flags[:], cps[:], KF, None, AL.is_gt)
            s16 = pool.tile([128, 1], f32, tag="s_s16", name=f"s16_{next(_n)}",
                            bufs=2)
            nc.vector.reduce_sum(s16[:],
                                 flags.rearrange("p (o j) -> p o j", o=1)[:],
                                 axis=AX.X)
            step = pool.tile([128, 1], f32, tag="s_step",
                             name=f"step_{next(_n)}", bufs=2)
            nc.vector.tensor_mul(step[:], s16[:], w8[:])
            lo2 = pool.tile([128, 1], f32, tag="s_lo", name=f"lo_{next(_n)}",
                            bufs=2)
            nc.vector.tensor_add(lo2[:], lo[:], step[:])
            hi2 = pool.tile([128, 1], f32, tag="s_hi", name=f"hi_{next(_n)}",
                            bufs=2)
            nc.vector.tensor_add(hi2[:], lo2[:], w8[:])
            lo, hi = lo2, hi2

        # global masked-out count -> kick collective early (overlaps thr/mask)
        cnt_f = mid("cnt_f", [128, 1])
        nc.vector.tensor_scalar(cjunk[:], g[:], hi[:], None, AL.is_gt,
                                op1=AL.add, accum_out=cnt_f[:])
        totp = psC([1, 1])
        nc.tensor.matmul(totp[:], cs["ones128"][:], cnt_f[:], start=True,
                         stop=True)
        tot_above = sbuf_copy(totp, "tot_above")
        cnt_i = dram.tile([1, 1], f32, tag="cnt_i")
        nc.sync.dma_start(cnt_i[:], tot_above[:])
        cnt_o = dram.tile([NCORES, 1], f32, tag="cnt_o")
        nc.gpsimd.collective_compute(
            "AllGather", AL.bypass, replica_groups=[list(range(NCORES))],
            ins=[cnt_i.opt()], outs=[cnt_o.opt()])

        # thr = rowmax(g where g <= hi); hi already replicated per partition
        selm = mid("selm", [128, 256], u8)
        nc.vector.tensor_scalar(selm[:], g[:], hi[:], None, AL.is_le)
        gm = mid("gmz")
        nc.gpsimd.memset(gm[:], -1.0e38)
        nc.vector.copy_predicated(gm[:], selm[:], g[:])
        gmx = mid("gmx", [128, 1])
        nc.vector.tensor_reduce(gmx[:], gm[:], axis=AX.X, op=AL.max)
        tpx = psB([1, 128])
        nc.tensor.transpose(tpx[:], gmx[:], ident[:])
        sx = sbuf_copy(tpx, "sx")
        redx = pool.tile([1, 16], f32, tag="redx")
        nc.vector.tensor_reduce(redx[:],
                                sx.rearrange("o (b q) -> o b q", q=8)[:],
                                axis=AX.X, op=AL.max)
        tpx2 = psB([16, 1])
        nc.tensor.transpose(tpx2[:], redx[:], ident[0:1, 0:1])
        thr16 = sbuf_copy(tpx2, "thr16")
        thrbc = psC([128, 1])
        nc.tensor.matmul(thrbc[:], cs["E16b"][:], thr16[:], start=True,
                         stop=True)
        mask01 = mid("mask01")
        nc.vector.tensor_scalar(mask01[:], g[:], thrbc[:], None, AL.is_le)

        # scale = 262144 / (262144 - total_above)
        allc = pool.tile([1, NCORES], f32, tag="allc")
        nc.sync.dma_start(allc[:], cnt_o.rearrange("r o -> o r")[:])
        tota = pool.tile([1, 1], f32, tag="tota")
        nc.vector.reduce_sum(tota[:],
                             allc.rearrange("o (a r) -> o a r", a=1)[:],
                             axis=AX.X)
        scl = pool.tile([1, 1], f32, tag="scl")
        nc.vector.tensor_scalar(scl[:], tota[:], -1.0, 262144.0, AL.mult,
                                op1=AL.add)
        nc.vector.reciprocal(scl[:], scl[:])
        nc.vector.tensor_scalar(scl[:], scl[:], 262144.0, None, AL.mult)
        sclbc = psC([128, 1])
        nc.tensor.matmul(sclbc[:], cs["ones1"][:], scl[:], start=True, stop=True)
        maskS = mid("maskS")
        nc.vector.tensor_scalar(maskS[:], mask01[:], sclbc[:], None, AL.mult)

        # mask columns: smT_h[p, b*8+q] = scaled mask at c' = q*256+h*128+p
        smt_list = []
        for h in range(2):
            tph = psB([128, 128])
            nc.tensor.transpose(tph[:], maskS[:, h * 128:(h + 1) * 128],
                                ident[:])
            sm = pool.tile([128, 128], f32, tag=f"smT{h}")
            nc.scalar.copy(sm[:], tph[:])
            smt_list.append(sm)

        # =========== PHASE C: mask-multiply from cache, store ===========
        ov = out_d.rearrange("b (p jj) h -> b p (jj h)", p=128)
        nhalf = 0
        for b in range(BL):
            for h in range(2):
                ot = xpool.tile([128, HFREE], f32, tag="xa")
                for jh in range(NJ // 2):
                    jj = h * 8 + jh
                    col = b * 8 + jj // 2
                    smcol = smt_list[jj % 2][:, col:col + 1]
                    src = cache[:,
                                b * FREE_B + jj * HW:b * FREE_B + (jj + 1) * HW]
                    dst = ot[:, jh * HW:(jh + 1) * HW]
                    if jj % 4 == 3:
                        nc.scalar.activation(dst, src, AF.Copy, scale=smcol)
                    else:
                        nc.vector.tensor_scalar(dst, src, smcol, None, AL.mult)
                dstv = ov[b, :, h * HFREE:(h + 1) * HFREE]
                if nhalf % 2 == 0:
                    nc.sync.dma_start(dstv, ot[:])
                else:
                    nc.gpsimd.dma_start(dstv, ot[:])
                nhalf += 1

    nc.finalize()
    return nc


def kernel(x, r, ratio, rho):
    x = np.ascontiguousarray(np.asarray(x, dtype=np.float32))
    r = np.ascontiguousarray(np.asarray(r, dtype=np.float32))
    ratio_f = float(np.asarray(ratio))
    rho_f = float(np.asarray(rho))
    k = int(ratio_f * C)
    key = (k, np.float32(rho_f).tobytes())
    if key not in _CACHE:
        _CACHE[key] = build(k, rho_f)
    nc = _CACHE[key]

    consts = _consts()
    xr = x.reshape(B, C, HW)
    in_maps = []
    for c in range(NCORES):
        envm = np.zeros((128, NCORES * 32), np.float32)
        e = c // 2
        envm[:, (2 * e) * 32:(2 * e + 2) * 32] = 1.0
        m = {"x": np.ascontiguousarray(xr[c * BL:(c + 1) * BL]),
             "r": np.ascontiguousarray(r[c * BL:(c + 1) * BL]),
             "envm": envm}
        m.update(consts)
        in_maps.append(m)
    res = run_bass_kernel_spmd(nc, in_maps, core_ids=list(range(NCORES)),
                               tmpdir=os.environ.get("BASS_TMPDIR"))
    LAST["res"] = res
    out = np.concatenate([res.results[c]["out"].reshape(BL, C, HW)
                          for c in range(NCORES)], axis=0)
    return out.reshape(B, C, 14, 14)
